# revision 3
# baseline (speedup 1.0000x reference)
"""MoE sigmoid routing (DeepSeek-V3 style noaux_tc) on 8 Trainium2 cores.

Token-major streaming: each slab is one 128-token tile's full 7168 dims,
so its PSUM accumulation finishes as soon as the slab lands and the
routing vector chain for tile t overlaps the DMA of tiles t+1..t+2.
Tail reorder work is done per-tile; only the final normalize + output
DMA trail the last slab.
"""
import numpy as np

import concourse.bacc as bacc
import concourse.mybir as mybir
import concourse.tile as tile
from concourse.bass_utils import run_bass_kernel_spmd

# problem constants (hardcoded per harness contract)
T, DIM, E, G, GW, TOPK = 16384, 7168, 256, 8, 32, 8
N_CORES = 8
T_LOC = T // N_CORES            # 2048 tokens per core
NT = T_LOC // 128               # 16 token tiles per core
KCH = DIM // 128                # 56 k-chunks
ROUTE_SCALE = 2.5
MM_DT = mybir.dt.float32r       # 1 cyc/row at N>=256; ~tf32 precision
NEG = -1e30

_CACHE = {}


def _build():
    nc = bacc.Bacc(None, target_bir_lowering=False)

    xp_d = nc.dram_tensor("xp", [NT, 128, KCH * 128], MM_DT,
                          kind="ExternalInput")
    wp_d = nc.dram_tensor("wp", [128, KCH * E], MM_DT, kind="ExternalInput")
    br_d = nc.dram_tensor("brep", [128, E], mybir.dt.float32,
                          kind="ExternalInput")
    wout_d = nc.dram_tensor("wout", [128, NT, TOPK],
                            mybir.dt.float32, kind="ExternalOutput")
    iout_d = nc.dram_tensor("iout", [128, NT, TOPK],
                            mybir.dt.int32, kind="ExternalOutput")

    with tile.TileContext(nc) as tc:
        with (
            tc.tile_pool(name="wpool", bufs=1) as wpool,
            tc.tile_pool(name="xpool", bufs=3) as xpool,
            tc.tile_pool(name="rpool", bufs=2) as rpool,
            tc.tile_pool(name="spool", bufs=2) as spool,
            tc.tile_pool(name="acc", bufs=1) as acc,
            tc.tile_pool(name="ps", bufs=1, space="PSUM") as ps,
        ):
            wt = wpool.tile([128, KCH * E], MM_DT, tag="wt")
            nc.sync.dma_start(out=wt[:], in_=wp_d[:])
            brep = wpool.tile([128, E], mybir.dt.float32, tag="brep")
            nc.sync.dma_start(out=brep[:], in_=br_d[:])

            idxu_all = acc.tile([128, NT * TOPK], mybir.dt.uint32, tag="idxu")
            sv_all = acc.tile([128, NT * TOPK], mybir.dt.float32, tag="sv")
            sxu_all = acc.tile([128, NT * TOPK], mybir.dt.uint32, tag="sxu")
            eq = acc.tile([128, NT * TOPK * 8], mybir.dt.float32, tag="eq")
            prod = acc.tile([128, NT * TOPK * 8], mybir.dt.float32, tag="prod")
            wsel = acc.tile([128, NT * TOPK], mybir.dt.float32, tag="wsel")
            rsum = acc.tile([128, NT], mybir.dt.float32, tag="rsum")
            rinv = acc.tile([128, NT], mybir.dt.float32, tag="rinv")
            wout = acc.tile([128, NT * TOPK], mybir.dt.float32, tag="wout")
            iout = acc.tile([128, NT * TOPK], mybir.dt.int32, tag="iout")

            KH = KCH // 4       # 14-chunk quarter-slabs: PE starts early,
            for t in range(NT):  # and the last-tile matmul tail shrinks
                xs = xpool.tile([128, KCH * 128], MM_DT, tag="xslab")
                for q in range(4):
                    nc.sync.dma_start(
                        out=xs[:, q * KH * 128:(q + 1) * KH * 128],
                        in_=xp_d[t, :, q * KH * 128:(q + 1) * KH * 128])
                psum = ps.tile([128, E], mybir.dt.float32, tag=f"ps{t % 4}",
                               name=f"psum_{t}")
                for c in range(KCH):
                    nc.tensor.matmul(
                        psum[:],
                        lhsT=xs[:, c * 128:(c + 1) * 128],
                        rhs=wt[:, c * E:(c + 1) * E],
                        start=(c == 0),
                        stop=(c == KCH - 1),
                    )

                sig = rpool.tile([128, E], mybir.dt.float32, tag="sig")
                nc.scalar.activation(sig[:], psum[:],
                                     mybir.ActivationFunctionType.Sigmoid)
                s = rpool.tile([128, E], mybir.dt.float32, tag="s")
                nc.vector.tensor_add(s[:], sig[:], brep[:])
                # group top-2 sum
                g1 = spool.tile([128, G], mybir.dt.float32, tag="g1")
                nc.vector.reduce_max(
                    g1[:], s[:].rearrange("p (g w) -> p g w", g=G),
                    axis=mybir.AxisListType.X)
                s2 = rpool.tile([128, E], mybir.dt.float32, tag="s2")
                nc.vector.match_replace(out=s2[:], in_to_replace=g1[:],
                                        in_values=s[:], imm_value=NEG)
                g2 = spool.tile([128, G], mybir.dt.float32, tag="g2")
                nc.vector.reduce_max(
                    g2[:], s2[:].rearrange("p (g w) -> p g w", g=G),
                    axis=mybir.AxisListType.X)
                gsc = spool.tile([128, G], mybir.dt.float32, tag="gsc")
                nc.vector.tensor_add(gsc[:], g1[:], g2[:])
                gsort = spool.tile([128, 8], mybir.dt.float32, tag="gsort")
                nc.vector.max(out=gsort[:], in_=gsc[:])
                gneg = spool.tile([128, G], mybir.dt.float32, tag="gneg")
                nc.vector.tensor_scalar(gneg[:], gsc[:], gsort[:, 3:4],
                                        NEG, op0=mybir.AluOpType.is_lt,
                                        op1=mybir.AluOpType.mult)
                m = rpool.tile([128, E], mybir.dt.float32, tag="m")
                nc.vector.tensor_add(
                    m[:].rearrange("p (g w) -> p g w", g=G),
                    s[:].rearrange("p (g w) -> p g w", g=G),
                    gneg[:].to_broadcast([128, G, GW]))
                # top-8 of masked s, in jax tie order
                v = spool.tile([128, 8], mybir.dt.float32, tag="v")
                nc.vector.max(out=v[:], in_=m[:])
                nc.vector.max_index(idxu_all[:, t * 8:(t + 1) * 8],
                                    v[:], m[:])
                # selected positions -> sigma values, sigma-rank order
                R = rpool.tile([128, E], mybir.dt.float32, tag="R")
                nc.vector.match_replace(out=R[:], in_to_replace=v[:],
                                        in_values=m[:], imm_value=NEG)
                selm = rpool.tile([128, E], mybir.dt.float32, tag="selm")
                nc.vector.tensor_tensor(selm[:], m[:], R[:],
                                        op=mybir.AluOpType.not_equal)
                sm = rpool.tile([128, E], mybir.dt.float32, tag="sm")
                nc.vector.tensor_mul(sm[:], sig[:], selm[:])
                nc.vector.max(out=sv_all[:, t * 8:(t + 1) * 8],
                              in_=sm[:])
                nc.vector.max_index(sxu_all[:, t * 8:(t + 1) * 8],
                                    sv_all[:, t * 8:(t + 1) * 8], sm[:])

                # per-tile reorder prep: sigma values into s-rank order via
                # 8x8 id match (overlaps later tiles' DMA)
                idxf = spool.tile([128, 8], mybir.dt.float32, tag="idxf")
                nc.vector.tensor_copy(idxf[:], idxu_all[:, t * 8:(t + 1) * 8])
                sxf = spool.tile([128, 8], mybir.dt.float32, tag="sxf")
                nc.vector.tensor_copy(sxf[:], sxu_all[:, t * 8:(t + 1) * 8])
                esl = eq[:, t * 64:(t + 1) * 64].rearrange(
                    "p (k j) -> p k j", k=TOPK)
                nc.vector.tensor_tensor(
                    esl,
                    idxf[:].to_broadcast([128, TOPK, TOPK]),
                    sxf[:].rearrange("p (a j) -> p a j", a=1).to_broadcast(
                        [128, TOPK, TOPK]),
                    op=mybir.AluOpType.is_equal)
                nc.vector.tensor_mul(
                    prod[:, t * 64:(t + 1) * 64].rearrange(
                        "p (k j) -> p k j", k=TOPK),
                    esl,
                    sv_all[:, t * 8:(t + 1) * 8].rearrange(
                        "p (a j) -> p a j", a=1).to_broadcast(
                        [128, TOPK, TOPK]))

                # per-tile fold + normalize + emit prep
                nc.vector.reduce_sum(
                    wsel[:, t * 8:(t + 1) * 8],
                    prod[:, t * 64:(t + 1) * 64].rearrange(
                        "p (tk j) -> p tk j", j=8),
                    axis=mybir.AxisListType.X)
                nc.vector.reduce_sum(
                    rsum[:, t:t + 1],
                    wsel[:, t * 8:(t + 1) * 8].rearrange(
                        "p (a k) -> p a k", a=1),
                    axis=mybir.AxisListType.X)
                nc.vector.reciprocal(rinv[:, t:t + 1], rsum[:, t:t + 1])
                nc.vector.tensor_scalar(
                    wout[:, t * 8:(t + 1) * 8],
                    wsel[:, t * 8:(t + 1) * 8],
                    rinv[:, t:t + 1], ROUTE_SCALE,
                    op0=mybir.AluOpType.mult,
                    op1=mybir.AluOpType.mult)
                nc.vector.tensor_copy(iout[:, t * 8:(t + 1) * 8],
                                      idxu_all[:, t * 8:(t + 1) * 8])

            nc.sync.dma_start(
                out=iout_d[:].rearrange("p t k -> p (t k)"), in_=iout[:])
            nc.sync.dma_start(
                out=wout_d[:].rearrange("p t k -> p (t k)"), in_=wout[:])

    nc.compile()
    return nc


def _prep_core_inputs(x_shard: np.ndarray, wp: np.ndarray,
                      brep: np.ndarray) -> dict:
    # xp[t, p, c*128 + j] = x_shard[t*128 + j, c*128 + p]
    v = x_shard.reshape(NT, 128, KCH, 128)          # [t, j, c, p]
    xp = np.ascontiguousarray(v.transpose(0, 3, 2, 1)).reshape(
        NT, 128, KCH * 128)
    return {"xp": xp, "wp": wp, "brep": brep}


def kernel(x: np.ndarray, weight: np.ndarray, bias: np.ndarray,
           _trace: bool = False):
    x = np.ascontiguousarray(np.asarray(x, dtype=np.float32))
    weight = np.ascontiguousarray(np.asarray(weight, dtype=np.float32))
    bias = np.ascontiguousarray(np.asarray(bias, dtype=np.float32))

    if "nc" not in _CACHE:
        _CACHE["nc"] = _build()
    nc = _CACHE["nc"]

    # wp[p, k*E + e] = weight[e, k*128 + p]
    wp = np.ascontiguousarray(
        weight.T.reshape(KCH, 128, E).transpose(1, 0, 2)).reshape(
        128, KCH * E)
    brep = np.ascontiguousarray(np.broadcast_to(bias, (128, E)))

    in_maps = [
        _prep_core_inputs(x[c * T_LOC:(c + 1) * T_LOC], wp, brep)
        for c in range(N_CORES)
    ]

    kw = {}
    if _trace:
        kw = {"trace": True}
    r = run_bass_kernel_spmd(nc, in_maps, core_ids=list(range(N_CORES)), **kw)
    _CACHE["last_result"] = r

    w_parts, i_parts = [], []
    for c in range(N_CORES):
        wo = r.results[c]["wout"]        # [128, NT, TOPK]
        io = r.results[c]["iout"]
        w_parts.append(wo.transpose(1, 0, 2).reshape(T_LOC, TOPK))
        i_parts.append(io.transpose(1, 0, 2).reshape(T_LOC, TOPK))
    weights_out = np.concatenate(w_parts, axis=0).astype(np.float32)
    indices_out = np.concatenate(i_parts, axis=0).astype(np.int32)
    return weights_out, indices_out


# revision 4
# speedup vs baseline: 1.0041x; 1.0041x over previous
"""MoE sigmoid routing (DeepSeek-V3 style noaux_tc) on 8 Trainium2 cores.

Token-major streaming: each slab is one 128-token tile's full 7168 dims,
so its PSUM accumulation finishes as soon as the slab lands and the
routing vector chain for tile t overlaps the DMA of tiles t+1..t+2.
Tail reorder work is done per-tile; only the final normalize + output
DMA trail the last slab.
"""
import numpy as np

import concourse.bacc as bacc
import concourse.mybir as mybir
import concourse.tile as tile
from concourse.bass_utils import run_bass_kernel_spmd

# problem constants (hardcoded per harness contract)
T, DIM, E, G, GW, TOPK = 16384, 7168, 256, 8, 32, 8
N_CORES = 8
T_LOC = T // N_CORES            # 2048 tokens per core
NT = T_LOC // 128               # 16 token tiles per core
KCH = DIM // 128                # 56 k-chunks
ROUTE_SCALE = 2.5
MM_DT = mybir.dt.float32r       # 1 cyc/row at N>=256; ~tf32 precision
NEG = -1e30

_CACHE = {}


def _build():
    nc = bacc.Bacc(None, target_bir_lowering=False)

    xp_d = nc.dram_tensor("xp", [NT, 128, KCH * 128], MM_DT,
                          kind="ExternalInput")
    wp_d = nc.dram_tensor("wp", [128, KCH * E], MM_DT, kind="ExternalInput")
    br_d = nc.dram_tensor("brep", [128, E], mybir.dt.float32,
                          kind="ExternalInput")
    wout_d = nc.dram_tensor("wout", [128, NT, TOPK],
                            mybir.dt.float32, kind="ExternalOutput")
    iout_d = nc.dram_tensor("iout", [128, NT, TOPK],
                            mybir.dt.int32, kind="ExternalOutput")

    with tile.TileContext(nc) as tc:
        with (
            tc.tile_pool(name="wpool", bufs=1) as wpool,
            tc.tile_pool(name="xpool", bufs=3) as xpool,
            tc.tile_pool(name="rpool", bufs=2) as rpool,
            tc.tile_pool(name="spool", bufs=2) as spool,
            tc.tile_pool(name="acc", bufs=1) as acc,
            tc.tile_pool(name="ps", bufs=1, space="PSUM") as ps,
        ):
            wt = wpool.tile([128, KCH * E], MM_DT, tag="wt")
            nc.sync.dma_start(out=wt[:], in_=wp_d[:])
            brep = wpool.tile([128, E], mybir.dt.float32, tag="brep")
            nc.sync.dma_start(out=brep[:], in_=br_d[:])

            idxu_all = acc.tile([128, NT * TOPK], mybir.dt.uint32, tag="idxu")
            sv_all = acc.tile([128, NT * TOPK], mybir.dt.float32, tag="sv")
            sxu_all = acc.tile([128, NT * TOPK], mybir.dt.uint32, tag="sxu")
            eq = acc.tile([128, NT * TOPK * 8], mybir.dt.float32, tag="eq")
            prod = acc.tile([128, NT * TOPK * 8], mybir.dt.float32, tag="prod")
            wsel = acc.tile([128, NT * TOPK], mybir.dt.float32, tag="wsel")
            rsum = acc.tile([128, NT], mybir.dt.float32, tag="rsum")
            rinv = acc.tile([128, NT], mybir.dt.float32, tag="rinv")
            wout = acc.tile([128, NT * TOPK], mybir.dt.float32, tag="wout")
            iout = acc.tile([128, NT * TOPK], mybir.dt.int32, tag="iout")

            KH = KCH // 8       # 7-chunk eighth-slabs: PE starts early,
            for t in range(NT):  # and the last-tile matmul tail shrinks
                xs = xpool.tile([128, KCH * 128], MM_DT, tag="xslab")
                for q in range(8):
                    nc.sync.dma_start(
                        out=xs[:, q * KH * 128:(q + 1) * KH * 128],
                        in_=xp_d[t, :, q * KH * 128:(q + 1) * KH * 128])
                psum = ps.tile([128, E], mybir.dt.float32, tag=f"ps{t % 4}",
                               name=f"psum_{t}")
                for c in range(KCH):
                    nc.tensor.matmul(
                        psum[:],
                        lhsT=xs[:, c * 128:(c + 1) * 128],
                        rhs=wt[:, c * E:(c + 1) * E],
                        start=(c == 0),
                        stop=(c == KCH - 1),
                    )

                sig = rpool.tile([128, E], mybir.dt.float32, tag="sig")
                nc.scalar.activation(sig[:], psum[:],
                                     mybir.ActivationFunctionType.Sigmoid)
                s = rpool.tile([128, E], mybir.dt.float32, tag="s")
                nc.vector.tensor_add(s[:], sig[:], brep[:])
                # group top-2 sum
                g1 = spool.tile([128, G], mybir.dt.float32, tag="g1")
                nc.vector.reduce_max(
                    g1[:], s[:].rearrange("p (g w) -> p g w", g=G),
                    axis=mybir.AxisListType.X)
                s2 = rpool.tile([128, E], mybir.dt.float32, tag="s2")
                nc.vector.match_replace(out=s2[:], in_to_replace=g1[:],
                                        in_values=s[:], imm_value=NEG)
                g2 = spool.tile([128, G], mybir.dt.float32, tag="g2")
                nc.vector.reduce_max(
                    g2[:], s2[:].rearrange("p (g w) -> p g w", g=G),
                    axis=mybir.AxisListType.X)
                gsc = spool.tile([128, G], mybir.dt.float32, tag="gsc")
                nc.vector.tensor_add(gsc[:], g1[:], g2[:])
                gsort = spool.tile([128, 8], mybir.dt.float32, tag="gsort")
                nc.vector.max(out=gsort[:], in_=gsc[:])
                gneg = spool.tile([128, G], mybir.dt.float32, tag="gneg")
                nc.vector.tensor_scalar(gneg[:], gsc[:], gsort[:, 3:4],
                                        NEG, op0=mybir.AluOpType.is_lt,
                                        op1=mybir.AluOpType.mult)
                m = rpool.tile([128, E], mybir.dt.float32, tag="m")
                nc.vector.tensor_add(
                    m[:].rearrange("p (g w) -> p g w", g=G),
                    s[:].rearrange("p (g w) -> p g w", g=G),
                    gneg[:].to_broadcast([128, G, GW]))
                # top-8 of masked s, in jax tie order
                v = spool.tile([128, 8], mybir.dt.float32, tag="v")
                nc.vector.max(out=v[:], in_=m[:])
                nc.vector.max_index(idxu_all[:, t * 8:(t + 1) * 8],
                                    v[:], m[:])
                nc.vector.tensor_copy(iout[:, t * 8:(t + 1) * 8],
                                      idxu_all[:, t * 8:(t + 1) * 8])
                # selected positions -> sigma values, sigma-rank order
                R = rpool.tile([128, E], mybir.dt.float32, tag="R")
                nc.vector.match_replace(out=R[:], in_to_replace=v[:],
                                        in_values=m[:], imm_value=NEG)
                selm = rpool.tile([128, E], mybir.dt.float32, tag="selm")
                nc.vector.tensor_tensor(selm[:], m[:], R[:],
                                        op=mybir.AluOpType.not_equal)
                sm = rpool.tile([128, E], mybir.dt.float32, tag="sm")
                nc.vector.tensor_mul(sm[:], sig[:], selm[:])
                nc.vector.max(out=sv_all[:, t * 8:(t + 1) * 8],
                              in_=sm[:])
                nc.vector.max_index(sxu_all[:, t * 8:(t + 1) * 8],
                                    sv_all[:, t * 8:(t + 1) * 8], sm[:])

                # per-tile reorder prep: sigma values into s-rank order via
                # 8x8 id match (overlaps later tiles' DMA)
                idxf = spool.tile([128, 8], mybir.dt.float32, tag="idxf")
                nc.vector.tensor_copy(idxf[:], idxu_all[:, t * 8:(t + 1) * 8])
                sxf = spool.tile([128, 8], mybir.dt.float32, tag="sxf")
                nc.vector.tensor_copy(sxf[:], sxu_all[:, t * 8:(t + 1) * 8])
                esl = eq[:, t * 64:(t + 1) * 64].rearrange(
                    "p (k j) -> p k j", k=TOPK)
                nc.vector.tensor_tensor(
                    esl,
                    idxf[:].to_broadcast([128, TOPK, TOPK]),
                    sxf[:].rearrange("p (a j) -> p a j", a=1).to_broadcast(
                        [128, TOPK, TOPK]),
                    op=mybir.AluOpType.is_equal)
                nc.vector.tensor_mul(
                    prod[:, t * 64:(t + 1) * 64].rearrange(
                        "p (k j) -> p k j", k=TOPK),
                    esl,
                    sv_all[:, t * 8:(t + 1) * 8].rearrange(
                        "p (a j) -> p a j", a=1).to_broadcast(
                        [128, TOPK, TOPK]))

                # per-tile fold + normalize + emit prep
                nc.vector.reduce_sum(
                    wsel[:, t * 8:(t + 1) * 8],
                    prod[:, t * 64:(t + 1) * 64].rearrange(
                        "p (tk j) -> p tk j", j=8),
                    axis=mybir.AxisListType.X)
                nc.vector.reduce_sum(
                    rsum[:, t:t + 1],
                    wsel[:, t * 8:(t + 1) * 8].rearrange(
                        "p (a k) -> p a k", a=1),
                    axis=mybir.AxisListType.X)
                nc.vector.reciprocal(rinv[:, t:t + 1], rsum[:, t:t + 1])
                nc.vector.tensor_scalar(
                    wout[:, t * 8:(t + 1) * 8],
                    wsel[:, t * 8:(t + 1) * 8],
                    rinv[:, t:t + 1], ROUTE_SCALE,
                    op0=mybir.AluOpType.mult,
                    op1=mybir.AluOpType.mult)

            nc.sync.dma_start(
                out=iout_d[:].rearrange("p t k -> p (t k)"), in_=iout[:])
            nc.sync.dma_start(
                out=wout_d[:].rearrange("p t k -> p (t k)"), in_=wout[:])

    nc.compile()
    return nc


def _prep_core_inputs(x_shard: np.ndarray, wp: np.ndarray,
                      brep: np.ndarray) -> dict:
    # xp[t, p, c*128 + j] = x_shard[t*128 + j, c*128 + p]
    v = x_shard.reshape(NT, 128, KCH, 128)          # [t, j, c, p]
    xp = np.ascontiguousarray(v.transpose(0, 3, 2, 1)).reshape(
        NT, 128, KCH * 128)
    return {"xp": xp, "wp": wp, "brep": brep}


def kernel(x: np.ndarray, weight: np.ndarray, bias: np.ndarray,
           _trace: bool = False):
    x = np.ascontiguousarray(np.asarray(x, dtype=np.float32))
    weight = np.ascontiguousarray(np.asarray(weight, dtype=np.float32))
    bias = np.ascontiguousarray(np.asarray(bias, dtype=np.float32))

    if "nc" not in _CACHE:
        _CACHE["nc"] = _build()
    nc = _CACHE["nc"]

    # wp[p, k*E + e] = weight[e, k*128 + p]
    wp = np.ascontiguousarray(
        weight.T.reshape(KCH, 128, E).transpose(1, 0, 2)).reshape(
        128, KCH * E)
    brep = np.ascontiguousarray(np.broadcast_to(bias, (128, E)))

    in_maps = [
        _prep_core_inputs(x[c * T_LOC:(c + 1) * T_LOC], wp, brep)
        for c in range(N_CORES)
    ]

    kw = {}
    if _trace:
        kw = {"trace": True}
    r = run_bass_kernel_spmd(nc, in_maps, core_ids=list(range(N_CORES)), **kw)
    _CACHE["last_result"] = r

    w_parts, i_parts = [], []
    for c in range(N_CORES):
        wo = r.results[c]["wout"]        # [128, NT, TOPK]
        io = r.results[c]["iout"]
        w_parts.append(wo.transpose(1, 0, 2).reshape(T_LOC, TOPK))
        i_parts.append(io.transpose(1, 0, 2).reshape(T_LOC, TOPK))
    weights_out = np.concatenate(w_parts, axis=0).astype(np.float32)
    indices_out = np.concatenate(i_parts, axis=0).astype(np.int32)
    return weights_out, indices_out
